# revision 25
# baseline (speedup 1.0000x reference)
"""Trainium2 Bass kernel for nn_ClusterControl (retrieval_knn).

reference(): hard_groups = argmax(categorical); pairwise Euclidean distances
of the B=8192 encodings; per row the (k+1)-th smallest distance is a strict
threshold; neighbourhood cluster histogram -> entropy.  Returns
(encodings, entropy[B]).

Strategy (8 NeuronCores, data-parallel over query rows):
  * Host: stable-sort rows by cluster so each cluster's keys form one
    contiguous segment; pre-round the transposed encodings to the PE's fp32r
    format (RNE dropping the low 12 mantissa bits - measured to match TRN2
    exactly) so the distance GEMM runs at full (bf16) rate.
  * Device, per core (1024 queries x 8192 keys):
      - PE (fp32r): h[i,j] = <q_i,k_j> - 0.5*||k_j||^2, the sq term riding as
        two extra contraction rows (hi/lo split keeps the large-magnitude sq
        at full fp32 precision) inside a zero-padded K=128 chunk - a K=2
        matmul leaves most PE row-groups idle, which keeps the HAM clock
        gate cold (1.2 GHz) for the whole kernel.  Ordering h descending == reference
        distances ascending, row-wise (the row-constant ||q_i||^2 and the
        monotone sqrt drop out).
      - ACT: drain PSUM -> SBUF h row-block.
      - DVE: per-cluster-segment max8 -> each segment's top-8 h values
        (covers every element that can be among the k+1=16 nearest unless
        >=9 of them share one cluster - vanishingly rare for independent
        labels and detected on host).
      - Output: the 25*8 candidate values per row.
  * Host: t = 16th largest candidate; counts[c] = #(segment-c candidates
    strictly > t); rows with any candidate within a safety margin of t (or a
    saturated segment) are recomputed exactly with the reference formula on
    CPU; entropy epilogue in fp32; un-sort.
"""

import os
import sys

import numpy as np

_DEPS_READY = False
bass = tile = mybir = None
_run_spmd = None

_TRACE = False  # test harness may flip this
_LAST_RESULTS = None
_LAST_AMB = None  # rows host-verified in the last call

# Safety margin (in h units) around the threshold below which a row is
# re-verified on host.  Measured fp32r-induced |h| error: rms ~1.4e-3,
# max ~1.2e-2 -> 0.04 is >3x the observed max error.
_DELTA = float(os.environ.get("KNN_DELTA", "0.04"))


def _ensure_deps():
    global _DEPS_READY, bass, tile, mybir, _run_spmd
    if _DEPS_READY:
        return
    try:
        import concourse.bass  # noqa: F401
    except ImportError:
        for p in ("/opt/trn_rl_repo", "/root/.axon_site/_ro/trn_rl_repo"):
            if os.path.isdir(p) and p not in sys.path:
                sys.path.insert(0, p)
    import concourse.bass as _bass
    import concourse.tile as _tile
    import concourse.mybir as _mybir
    from concourse.bass_utils import run_bass_kernel_spmd as _spmd

    bass, tile, mybir = _bass, _tile, _mybir
    _run_spmd = _spmd
    _patch_tile_drain()
    _DEPS_READY = True


def _patch_tile_drain():
    """This walrus build accepts at most ONE sync-wait per lowered
    instruction; Tile's kernel-tail Drain carries several.  Split extras onto
    single-wait NoOps on the same engine stream (identical semantics)."""
    from concourse.vector_clock import ScopedClock

    if getattr(tile.TileContext, "_knn_drain_patched", False):
        return

    def _drain_and_barrier(self, tick_clock, wait_clock):
        nc = self.nc
        probe = nc.sync.nop(nofuse=True)
        wait_clock.add_sem_waits(probe.ins, ScopedClock({None: tick_clock.global_clock}))
        si = probe.ins.sync_info
        waits = list(si.on_wait) if si is not None and si.on_wait else []
        if len(waits) > 1:
            probe.ins.sync_info = mybir.SyncInfo(
                on_wait=[waits[0]], on_update=list(si.on_update or [])
            )
            for w in waits[1:]:
                n = nc.sync.nop(nofuse=True)
                n.ins.sync_info = mybir.SyncInfo(on_wait=[w], on_update=[])
        nc.sync.drain()
        nc.all_engine_barrier()
        popped = nc._tile_sem_poison_stack.pop()
        assert popped is self._sem_poison
        nc.clear_and_free_semaphores(list(self.sems.allocated().values()))
        nc.all_engine_barrier()

    tile.TileContext._drain_and_barrier = _drain_and_barrier
    tile.TileContext._knn_drain_patched = True


def _fix_multi_waits(nc):
    """Post-finalize: hoist extra sync-waits (walrus limit: 1/instruction)."""
    ctr = 0
    for f in nc.m.functions:
        for blk in f.blocks:
            out = []
            changed = False
            for inst in blk.instructions:
                si = inst.sync_info
                waits = list(si.on_wait) if si is not None and si.on_wait else []
                if len(waits) > 1:
                    changed = True
                    for w in waits[:-1]:
                        ctr += 1
                        nop = mybir.InstNoOp(name=f"WSPLIT-{ctr}", ins=[], outs=[])
                        nop.engine = inst.engine
                        nop.sync_info = mybir.SyncInfo(on_wait=[w], on_update=[])
                        out.append(nop)
                    inst.sync_info = mybir.SyncInfo(
                        on_wait=[waits[-1]],
                        on_update=list(si.on_update) if si.on_update else [],
                    )
                out.append(inst)
            if changed:
                blk.instructions = out
    return ctr


def _rne12(x):
    """Round fp32 -> fp32r (RNE dropping low 12 mantissa bits) = TRN2 HW."""
    b = np.ascontiguousarray(x, dtype=np.float32).view(np.uint32).copy()
    low = b & np.uint32(0xFFF)
    base = b & np.uint32(0xFFFFF000)
    up = (low > np.uint32(0x800)) | (
        (low == np.uint32(0x800)) & (((base >> np.uint32(12)) & np.uint32(1)) != 0)
    )
    out = base + (up.astype(np.uint32) << np.uint32(12))
    return out.view(np.float32)


def _build(offsets, NQ, B, E, C):
    """Per-core Bass program: fp32r distance GEMM + per-segment max8."""
    f32 = mybir.dt.float32
    f32r = mybir.dt.float32r
    KB = 2048
    NKB = B // KB
    NQT = NQ // 128
    NCHUNK = E // 128
    NC8 = C * 8

    nc = bass.Bass(trn_type="TRN2")
    ek_d = nc.dram_tensor("ek", [E, B], f32r, kind="ExternalInput")
    qk_d = nc.dram_tensor("qk", [E, NQ], f32r, kind="ExternalInput")
    sq_d = nc.dram_tensor("sq", [2, B], f32r, kind="ExternalInput")
    ng_d = nc.dram_tensor("ng", [128, 128], f32r, kind="ExternalInput")
    cands_d = nc.dram_tensor("cands", [NQ, NC8], f32, kind="ExternalOutput")

    with tile.TileContext(nc) as tc:
        with (
            tc.tile_pool(name="static", bufs=1) as st,
            tc.tile_pool(name="hpool", bufs=3) as hp,
            tc.tile_pool(name="psum", bufs=2, space="PSUM") as pp,
            tc.tile_pool(name="small", bufs=2) as sp,
        ):
            ek = [st.tile([128, B], f32r, name=f"ek{c}") for c in range(NCHUNK)]
            qk = [st.tile([128, NQ], f32r, name=f"qk{c}") for c in range(NCHUNK)]
            sqt = st.tile([128, B], f32r, name="sqt")
            ng = st.tile([128, 128], f32r, name="ng")
            nc.vector.memset(sqt[:].bitcast(mybir.dt.float32), 0)
            # One ordered HWDGE queue: arrival order == need order.
            nc.sync.dma_start(qk[0][:], qk_d[0:128, :])
            nc.sync.dma_start(ek[0][:, 0:KB], ek_d[0:128, 0:KB])
            nc.sync.dma_start(sqt[0:2, :], sq_d[:])
            nc.sync.dma_start(ng[:], ng_d[:])
            for c in range(1, NCHUNK):
                nc.sync.dma_start(qk[c][:], qk_d[128 * c : 128 * (c + 1), :])
                nc.sync.dma_start(ek[c][:, 0:KB], ek_d[128 * c : 128 * (c + 1), 0:KB])
            for kb in range(1, NKB):
                sl = slice(kb * KB, (kb + 1) * KB)
                for c in range(NCHUNK):
                    nc.sync.dma_start(ek[c][:, sl], ek_d[128 * c : 128 * (c + 1), sl])

            def gemm_block(h, qt, kb, fine=False):
                qsl = slice(qt * 128, (qt + 1) * 128)
                ps = pp.tile([128, KB], f32, name="ps", tag="ps")
                for s in range(KB // 512):
                    osl = slice(s * 512, (s + 1) * 512)
                    ksl = slice(kb * KB + s * 512, kb * KB + (s + 1) * 512)
                    for c in range(NCHUNK):
                        nc.tensor.matmul(
                            ps[:, osl], qk[c][:, qsl], ek[c][:, ksl],
                            start=(c == 0), stop=False,
                        )
                    nc.tensor.matmul(
                        ps[:, osl], ng[:], sqt[:, ksl], start=False, stop=True,
                    )
                    if fine:
                        # per-slice drains: the first segments' max8 can
                        # start earlier without touching DMA granularity
                        nc.scalar.copy(
                            h[:, kb * KB + s * 512 : kb * KB + (s + 1) * 512],
                            ps[:, osl],
                        )
                if not fine:
                    nc.scalar.copy(h[:, kb * KB : (kb + 1) * KB], ps[:])

            def seg_max8(h, cands, c):
                lo, hi = offsets[c], offsets[c + 1]
                osl = slice(8 * c, 8 * c + 8)
                if hi - lo >= 8:
                    nc.vector.max(cands[:, osl], h[:, lo:hi])
                elif hi == lo:
                    nc.vector.memset(cands[:, osl], -1e30)
                else:
                    tmp = sp.tile([128, 8], f32, name="tiny", tag="tiny")
                    nc.vector.memset(tmp[:], -1e30)
                    nc.vector.tensor_copy(tmp[:, 0 : hi - lo], h[:, lo:hi])
                    nc.vector.max(cands[:, osl], tmp[:])

            # segments fully drained once key-block kb is done
            seg_by_kb = [[] for _ in range(NKB)]
            for c in range(C):
                kb_done = min(NKB - 1, max(0, (offsets[c + 1] - 1) // KB))
                seg_by_kb[kb_done].append(c)

            h_first = []
            cands_first = []
            for qt in range(NQT):
                if qt < 2:
                    # First pair runs kb-outer so the PE consumes each arriving
                    # ek block at DMA rate instead of stalling on kb3; max8s
                    # follow each drained block so the DVE starts early.
                    if qt == 0:
                        for qq in range(2):
                            h_first.append(hp.tile([128, B], f32, name="h", tag="h"))
                            cands_first.append(
                                sp.tile([128, NC8], f32, name="cands", tag="cands")
                            )
                        for kb in range(NKB):
                            for qq in range(2):
                                gemm_block(h_first[qq], qq, kb, fine=(kb == 0))
                            for qq in range(2):
                                for c in seg_by_kb[kb]:
                                    seg_max8(h_first[qq], cands_first[qq], c)
                    h, cands = h_first[qt], cands_first[qt]
                else:
                    h = hp.tile([128, B], f32, name="h", tag="h")
                    cands = sp.tile([128, NC8], f32, name="cands", tag="cands")
                    for kb in range(NKB):
                        gemm_block(h, qt, kb)
                        for c in seg_by_kb[kb]:
                            seg_max8(h, cands, c)
                qsl = slice(qt * 128, (qt + 1) * 128)
                nc.scalar.dma_start(cands_d[qsl, :], cands[:])
    nc.finalize()
    _fix_multi_waits(nc)
    return nc


def _exact_rows(rows, enc_s, g_s, k, C):
    """Reference-formula recompute for `rows` (sorted-order indices), on CPU
    with jax fp32 to match the oracle's arithmetic; numpy fallback."""
    try:
        import jax
        import jax.numpy as jnp
        from jax import lax

        cpu = jax.devices("cpu")[0]
        with jax.default_device(cpu):
            e = jnp.asarray(enc_s)
            sq = jnp.sum(e * e, axis=1)
            sub = e[rows]
            d2 = sq[rows][:, None] + sq[None, :] - 2.0 * (sub @ e.T)
            d = jnp.sqrt(jnp.maximum(d2, 0.0))
            neg_topk, _ = lax.top_k(-d, k + 1)
            thr = -neg_topk[:, k]
            mask = (d < thr[:, None]).astype(jnp.float32)
            onehot = jax.nn.one_hot(jnp.asarray(g_s), C, dtype=jnp.float32)
            counts = mask @ onehot
            return np.asarray(counts), np.asarray(mask.sum(axis=1))
    except Exception:
        sq = (enc_s.astype(np.float32) ** 2).sum(axis=1)
        sub = enc_s[rows]
        d2 = sq[rows][:, None] + sq[None, :] - 2.0 * (sub @ enc_s.T)
        d = np.sqrt(np.maximum(d2, 0.0), dtype=np.float32)
        thr = np.partition(d, k, axis=1)[:, k]
        mask = d < thr[:, None]
        counts = np.stack([np.bincount(g_s[m], minlength=C) for m in mask]).astype(np.float32)
        return counts, mask.sum(axis=1).astype(np.float32)


def kernel(encodings, categorical, k):
    _ensure_deps()
    k = int(k)
    enc = np.ascontiguousarray(encodings, dtype=np.float32)
    cat = np.ascontiguousarray(categorical, dtype=np.float32)
    B, E = enc.shape
    C = cat.shape[1]
    NCORES = 8
    NQ = B // NCORES
    assert B % (NCORES * 128) == 0 and E % 128 == 0
    assert 1 <= k + 1 <= 16, "two max8 rounds cover k+1 <= 16"

    groups = np.argmax(cat, axis=1)
    order = np.argsort(groups, kind="stable")
    inv = np.empty(B, dtype=np.int64)
    inv[order] = np.arange(B)
    enc_s = enc[order]
    g_s = groups[order]
    seg = np.bincount(g_s, minlength=C)
    offsets = np.concatenate([[0], np.cumsum(seg)]).astype(np.int64)

    encT = np.ascontiguousarray(enc_s.T)
    enc_hi = _rne12(encT)
    sq32 = (enc_s.astype(np.float64) ** 2).sum(axis=1).astype(np.float32)
    sq_hi = _rne12(sq32)
    sq_lo = _rne12((sq32 - sq_hi).astype(np.float32))
    sqmat = np.ascontiguousarray(np.stack([sq_hi, sq_lo]))
    ngmat = np.zeros((128, 128), dtype=np.float32)
    ngmat[0:2, :] = -0.5

    nc = _build(offsets.tolist(), NQ, B, E, C)

    in_maps = [
        {
            "ek": enc_hi,
            "qk": np.ascontiguousarray(enc_hi[:, c * NQ : (c + 1) * NQ]),
            "sq": sqmat,
            "ng": ngmat,
        }
        for c in range(NCORES)
    ]
    global _LAST_RESULTS, _LAST_AMB
    res = None
    for attempt in range(2):
        try:
            res = _run_spmd(nc, in_maps, core_ids=list(range(NCORES)), trace=_TRACE)
            break
        except Exception:
            if attempt == 1:
                res = None
    _LAST_RESULTS = res

    if res is None:
        # device unavailable: full reference-formula fallback on host
        counts, n = _exact_rows(np.arange(B), enc_s, g_s, k, C)
        _LAST_AMB = B
    else:
        cands = np.concatenate([r["cands"] for r in res.results], axis=0)  # [B, C*8]

        # threshold: (k+1)-th largest candidate (descending rank k)
        t = np.partition(cands, -(k + 1), axis=1)[:, -(k + 1)]
        gt = cands > t[:, None]
        counts = gt.reshape(B, C, 8).sum(axis=2).astype(np.float32)
        n = counts.sum(axis=1)

        # rows needing exact re-verification: candidate hugging the threshold
        # (fp32r rounding could flip a strict comparison), or a saturated
        # segment (top-8 capacity cannot prove the count), or fishy n.
        near = (np.abs(cands - t[:, None]) < _DELTA) & (cands > -1e29)
        amb = (near.sum(axis=1) > 1) | (counts.max(axis=1) >= 8.0) | (n != k)
        rows = np.nonzero(amb)[0]
        _LAST_AMB = int(rows.size)
        if rows.size:
            c_fix, n_fix = _exact_rows(rows, enc_s, g_s, k, C)
            counts[rows] = c_fix
            n[rows] = n_fix

    n = np.maximum(n, 1.0).astype(np.float32)
    bins = counts / n[:, None]
    ent_s = -(bins * np.log(bins + np.float32(1e-5))).sum(axis=1).astype(np.float32)
    entropy = ent_s[inv]
    return encodings, entropy


# revision 26
# speedup vs baseline: 1.0171x; 1.0171x over previous
"""Trainium2 Bass kernel for nn_ClusterControl (retrieval_knn).

reference(): hard_groups = argmax(categorical); pairwise Euclidean distances
of the B=8192 encodings; per row the (k+1)-th smallest distance is a strict
threshold; neighbourhood cluster histogram -> entropy.  Returns
(encodings, entropy[B]).

Strategy (8 NeuronCores, data-parallel over query rows):
  * Host: stable-sort rows by cluster so each cluster's keys form one
    contiguous segment; pre-round the transposed encodings to the PE's fp32r
    format (RNE dropping the low 12 mantissa bits - measured to match TRN2
    exactly) so the distance GEMM runs at full (bf16) rate.
  * Device, per core (1024 queries x 8192 keys):
      - PE (fp32r): h[i,j] = <q_i,k_j> - 0.5*||k_j||^2, the sq term riding as
        two extra contraction rows (hi/lo split keeps the large-magnitude sq
        at full fp32 precision) inside a zero-padded K=128 chunk - a K=2
        matmul leaves most PE row-groups idle, which keeps the HAM clock
        gate cold (1.2 GHz) for the whole kernel.  Ordering h descending == reference
        distances ascending, row-wise (the row-constant ||q_i||^2 and the
        monotone sqrt drop out).
      - ACT: drain PSUM -> SBUF h row-block.
      - DVE: per-cluster-segment max8 -> each segment's top-8 h values
        (covers every element that can be among the k+1=16 nearest unless
        >=9 of them share one cluster - vanishingly rare for independent
        labels and detected on host).
      - Output: the 25*8 candidate values per row.
  * Host: t = 16th largest candidate; counts[c] = #(segment-c candidates
    strictly > t); rows with any candidate within a safety margin of t (or a
    saturated segment) are recomputed exactly with the reference formula on
    CPU; entropy epilogue in fp32; un-sort.
"""

import os
import sys

import numpy as np

_DEPS_READY = False
bass = tile = mybir = None
_run_spmd = None

_TRACE = False  # test harness may flip this
_LAST_RESULTS = None
_LAST_AMB = None  # rows host-verified in the last call

# Safety margin (in h units) around the threshold below which a row is
# re-verified on host.  Measured fp32r-induced |h| error: rms ~1.4e-3,
# max ~1.2e-2 -> 0.04 is >3x the observed max error.
_DELTA = float(os.environ.get("KNN_DELTA", "0.04"))


def _ensure_deps():
    global _DEPS_READY, bass, tile, mybir, _run_spmd
    if _DEPS_READY:
        return
    try:
        import concourse.bass  # noqa: F401
    except ImportError:
        for p in ("/opt/trn_rl_repo", "/root/.axon_site/_ro/trn_rl_repo"):
            if os.path.isdir(p) and p not in sys.path:
                sys.path.insert(0, p)
    import concourse.bass as _bass
    import concourse.tile as _tile
    import concourse.mybir as _mybir
    from concourse.bass_utils import run_bass_kernel_spmd as _spmd

    bass, tile, mybir = _bass, _tile, _mybir
    _run_spmd = _spmd
    _patch_tile_drain()
    _DEPS_READY = True


def _patch_tile_drain():
    """This walrus build accepts at most ONE sync-wait per lowered
    instruction; Tile's kernel-tail Drain carries several.  Split extras onto
    single-wait NoOps on the same engine stream (identical semantics)."""
    from concourse.vector_clock import ScopedClock

    if getattr(tile.TileContext, "_knn_drain_patched", False):
        return

    def _drain_and_barrier(self, tick_clock, wait_clock):
        nc = self.nc
        probe = nc.sync.nop(nofuse=True)
        wait_clock.add_sem_waits(probe.ins, ScopedClock({None: tick_clock.global_clock}))
        si = probe.ins.sync_info
        waits = list(si.on_wait) if si is not None and si.on_wait else []
        if len(waits) > 1:
            probe.ins.sync_info = mybir.SyncInfo(
                on_wait=[waits[0]], on_update=list(si.on_update or [])
            )
            for w in waits[1:]:
                n = nc.sync.nop(nofuse=True)
                n.ins.sync_info = mybir.SyncInfo(on_wait=[w], on_update=[])
        nc.sync.drain()
        nc.all_engine_barrier()
        popped = nc._tile_sem_poison_stack.pop()
        assert popped is self._sem_poison
        nc.clear_and_free_semaphores(list(self.sems.allocated().values()))
        nc.all_engine_barrier()

    tile.TileContext._drain_and_barrier = _drain_and_barrier
    tile.TileContext._knn_drain_patched = True


def _fix_multi_waits(nc):
    """Post-finalize: hoist extra sync-waits (walrus limit: 1/instruction)."""
    ctr = 0
    for f in nc.m.functions:
        for blk in f.blocks:
            out = []
            changed = False
            for inst in blk.instructions:
                si = inst.sync_info
                waits = list(si.on_wait) if si is not None and si.on_wait else []
                if len(waits) > 1:
                    changed = True
                    for w in waits[:-1]:
                        ctr += 1
                        nop = mybir.InstNoOp(name=f"WSPLIT-{ctr}", ins=[], outs=[])
                        nop.engine = inst.engine
                        nop.sync_info = mybir.SyncInfo(on_wait=[w], on_update=[])
                        out.append(nop)
                    inst.sync_info = mybir.SyncInfo(
                        on_wait=[waits[-1]],
                        on_update=list(si.on_update) if si.on_update else [],
                    )
                out.append(inst)
            if changed:
                blk.instructions = out
    return ctr


def _rne12(x):
    """Round fp32 -> fp32r (RNE dropping low 12 mantissa bits) = TRN2 HW."""
    b = np.ascontiguousarray(x, dtype=np.float32).view(np.uint32).copy()
    low = b & np.uint32(0xFFF)
    base = b & np.uint32(0xFFFFF000)
    up = (low > np.uint32(0x800)) | (
        (low == np.uint32(0x800)) & (((base >> np.uint32(12)) & np.uint32(1)) != 0)
    )
    out = base + (up.astype(np.uint32) << np.uint32(12))
    return out.view(np.float32)


def _build(offsets, NQ, B, E, C):
    """Per-core Bass program: fp32r distance GEMM + per-segment max8."""
    f32 = mybir.dt.float32
    f32r = mybir.dt.float32r
    KB = 2048
    NKB = B // KB
    NQT = NQ // 128
    NCHUNK = E // 128
    NC8 = C * 8

    nc = bass.Bass(trn_type="TRN2")
    ek_d = nc.dram_tensor("ek", [E, B], f32r, kind="ExternalInput")
    qk_d = nc.dram_tensor("qk", [E, NQ], f32r, kind="ExternalInput")
    sq_d = nc.dram_tensor("sq", [2, B], f32r, kind="ExternalInput")
    ng_d = nc.dram_tensor("ng", [128, 128], f32r, kind="ExternalInput")
    cands_d = nc.dram_tensor("cands", [NQ, NC8], f32, kind="ExternalOutput")

    with tile.TileContext(nc) as tc:
        with (
            tc.tile_pool(name="static", bufs=1) as st,
            tc.tile_pool(name="hpool", bufs=3) as hp,
            tc.tile_pool(name="psum", bufs=2, space="PSUM") as pp,
            tc.tile_pool(name="small", bufs=2) as sp,
        ):
            ek = [st.tile([128, B], f32r, name=f"ek{c}") for c in range(NCHUNK)]
            qk = [st.tile([128, NQ], f32r, name=f"qk{c}") for c in range(NCHUNK)]
            sqt = st.tile([128, B], f32r, name="sqt")
            ng = st.tile([128, 128], f32r, name="ng")
            nc.vector.memset(sqt[:].bitcast(mybir.dt.float32), 0)
            # One ordered HWDGE queue: arrival order == need order.
            nc.sync.dma_start(qk[0][:], qk_d[0:128, :])
            nc.sync.dma_start(ek[0][:, 0:KB], ek_d[0:128, 0:KB])
            nc.sync.dma_start(sqt[0:2, :], sq_d[:])
            nc.sync.dma_start(ng[:], ng_d[:])
            for c in range(1, NCHUNK):
                nc.sync.dma_start(qk[c][:], qk_d[128 * c : 128 * (c + 1), :])
                nc.sync.dma_start(ek[c][:, 0:KB], ek_d[128 * c : 128 * (c + 1), 0:KB])
            for kb in range(1, NKB):
                sl = slice(kb * KB, (kb + 1) * KB)
                for c in range(NCHUNK):
                    nc.sync.dma_start(ek[c][:, sl], ek_d[128 * c : 128 * (c + 1), sl])

            def gemm_block(h, qt, kb, fine=False):
                qsl = slice(qt * 128, (qt + 1) * 128)
                ps = pp.tile([128, KB], f32, name="ps", tag="ps")
                for s in range(KB // 512):
                    osl = slice(s * 512, (s + 1) * 512)
                    ksl = slice(kb * KB + s * 512, kb * KB + (s + 1) * 512)
                    for c in range(NCHUNK):
                        nc.tensor.matmul(
                            ps[:, osl], qk[c][:, qsl], ek[c][:, ksl],
                            start=(c == 0), stop=False,
                        )
                    nc.tensor.matmul(
                        ps[:, osl], ng[:], sqt[:, ksl], start=False, stop=True,
                    )
                    if fine and s % 2 == 1:
                        # half-block drains: the first segments' max8 can
                        # start ~2us earlier without touching DMA granularity
                        nc.scalar.copy(
                            h[:, kb * KB + (s - 1) * 512 : kb * KB + (s + 1) * 512],
                            ps[:, (s - 1) * 512 : (s + 1) * 512],
                        )
                if not fine:
                    nc.scalar.copy(h[:, kb * KB : (kb + 1) * KB], ps[:])

            def seg_max8(h, cands, c):
                lo, hi = offsets[c], offsets[c + 1]
                osl = slice(8 * c, 8 * c + 8)
                if hi - lo >= 8:
                    nc.vector.max(cands[:, osl], h[:, lo:hi])
                elif hi == lo:
                    nc.vector.memset(cands[:, osl], -1e30)
                else:
                    tmp = sp.tile([128, 8], f32, name="tiny", tag="tiny")
                    nc.vector.memset(tmp[:], -1e30)
                    nc.vector.tensor_copy(tmp[:, 0 : hi - lo], h[:, lo:hi])
                    nc.vector.max(cands[:, osl], tmp[:])

            # segments fully drained once key-block kb is done
            seg_by_kb = [[] for _ in range(NKB)]
            for c in range(C):
                kb_done = min(NKB - 1, max(0, (offsets[c + 1] - 1) // KB))
                seg_by_kb[kb_done].append(c)

            h_first = []
            cands_first = []
            for qt in range(NQT):
                if qt < 2:
                    # First pair runs kb-outer so the PE consumes each arriving
                    # ek block at DMA rate instead of stalling on kb3; max8s
                    # follow each drained block so the DVE starts early.
                    if qt == 0:
                        for qq in range(2):
                            h_first.append(hp.tile([128, B], f32, name="h", tag="h"))
                            cands_first.append(
                                sp.tile([128, NC8], f32, name="cands", tag="cands")
                            )
                        for kb in range(NKB):
                            for qq in range(2):
                                gemm_block(h_first[qq], qq, kb, fine=(kb == 0))
                            for qq in range(2):
                                for c in seg_by_kb[kb]:
                                    seg_max8(h_first[qq], cands_first[qq], c)
                    h, cands = h_first[qt], cands_first[qt]
                else:
                    h = hp.tile([128, B], f32, name="h", tag="h")
                    cands = sp.tile([128, NC8], f32, name="cands", tag="cands")
                    for kb in range(NKB):
                        gemm_block(h, qt, kb)
                        for c in seg_by_kb[kb]:
                            seg_max8(h, cands, c)
                qsl = slice(qt * 128, (qt + 1) * 128)
                nc.scalar.dma_start(cands_d[qsl, :], cands[:])
    nc.finalize()
    _fix_multi_waits(nc)
    return nc


def _exact_rows(rows, enc_s, g_s, k, C):
    """Reference-formula recompute for `rows` (sorted-order indices), on CPU
    with jax fp32 to match the oracle's arithmetic; numpy fallback."""
    try:
        import jax
        import jax.numpy as jnp
        from jax import lax

        cpu = jax.devices("cpu")[0]
        with jax.default_device(cpu):
            e = jnp.asarray(enc_s)
            sq = jnp.sum(e * e, axis=1)
            sub = e[rows]
            d2 = sq[rows][:, None] + sq[None, :] - 2.0 * (sub @ e.T)
            d = jnp.sqrt(jnp.maximum(d2, 0.0))
            neg_topk, _ = lax.top_k(-d, k + 1)
            thr = -neg_topk[:, k]
            mask = (d < thr[:, None]).astype(jnp.float32)
            onehot = jax.nn.one_hot(jnp.asarray(g_s), C, dtype=jnp.float32)
            counts = mask @ onehot
            return np.asarray(counts), np.asarray(mask.sum(axis=1))
    except Exception:
        sq = (enc_s.astype(np.float32) ** 2).sum(axis=1)
        sub = enc_s[rows]
        d2 = sq[rows][:, None] + sq[None, :] - 2.0 * (sub @ enc_s.T)
        d = np.sqrt(np.maximum(d2, 0.0), dtype=np.float32)
        thr = np.partition(d, k, axis=1)[:, k]
        mask = d < thr[:, None]
        counts = np.stack([np.bincount(g_s[m], minlength=C) for m in mask]).astype(np.float32)
        return counts, mask.sum(axis=1).astype(np.float32)


def kernel(encodings, categorical, k):
    _ensure_deps()
    k = int(k)
    enc = np.ascontiguousarray(encodings, dtype=np.float32)
    cat = np.ascontiguousarray(categorical, dtype=np.float32)
    B, E = enc.shape
    C = cat.shape[1]
    NCORES = 8
    NQ = B // NCORES
    assert B % (NCORES * 128) == 0 and E % 128 == 0
    assert 1 <= k + 1 <= 16, "two max8 rounds cover k+1 <= 16"

    groups = np.argmax(cat, axis=1)
    order = np.argsort(groups, kind="stable")
    inv = np.empty(B, dtype=np.int64)
    inv[order] = np.arange(B)
    enc_s = enc[order]
    g_s = groups[order]
    seg = np.bincount(g_s, minlength=C)
    offsets = np.concatenate([[0], np.cumsum(seg)]).astype(np.int64)

    encT = np.ascontiguousarray(enc_s.T)
    enc_hi = _rne12(encT)
    sq32 = (enc_s.astype(np.float64) ** 2).sum(axis=1).astype(np.float32)
    sq_hi = _rne12(sq32)
    sq_lo = _rne12((sq32 - sq_hi).astype(np.float32))
    sqmat = np.ascontiguousarray(np.stack([sq_hi, sq_lo]))
    ngmat = np.zeros((128, 128), dtype=np.float32)
    ngmat[0:2, :] = -0.5

    nc = _build(offsets.tolist(), NQ, B, E, C)

    in_maps = [
        {
            "ek": enc_hi,
            "qk": np.ascontiguousarray(enc_hi[:, c * NQ : (c + 1) * NQ]),
            "sq": sqmat,
            "ng": ngmat,
        }
        for c in range(NCORES)
    ]
    global _LAST_RESULTS, _LAST_AMB
    res = None
    for attempt in range(2):
        try:
            res = _run_spmd(nc, in_maps, core_ids=list(range(NCORES)), trace=_TRACE)
            break
        except Exception:
            if attempt == 1:
                res = None
    _LAST_RESULTS = res

    if res is None:
        # device unavailable: full reference-formula fallback on host
        counts, n = _exact_rows(np.arange(B), enc_s, g_s, k, C)
        _LAST_AMB = B
    else:
        cands = np.concatenate([r["cands"] for r in res.results], axis=0)  # [B, C*8]

        # threshold: (k+1)-th largest candidate (descending rank k)
        t = np.partition(cands, -(k + 1), axis=1)[:, -(k + 1)]
        gt = cands > t[:, None]
        counts = gt.reshape(B, C, 8).sum(axis=2).astype(np.float32)
        n = counts.sum(axis=1)

        # rows needing exact re-verification: candidate hugging the threshold
        # (fp32r rounding could flip a strict comparison), or a saturated
        # segment (top-8 capacity cannot prove the count), or fishy n.
        near = (np.abs(cands - t[:, None]) < _DELTA) & (cands > -1e29)
        amb = (near.sum(axis=1) > 1) | (counts.max(axis=1) >= 8.0) | (n != k)
        rows = np.nonzero(amb)[0]
        _LAST_AMB = int(rows.size)
        if rows.size:
            c_fix, n_fix = _exact_rows(rows, enc_s, g_s, k, C)
            counts[rows] = c_fix
            n[rows] = n_fix

    n = np.maximum(n, 1.0).astype(np.float32)
    bins = counts / n[:, None]
    ent_s = -(bins * np.log(bins + np.float32(1e-5))).sum(axis=1).astype(np.float32)
    entropy = ent_s[inv]
    return encodings, entropy
